# revision 54
# baseline (speedup 1.0000x reference)
"""Bass/Tile kernel for nn_EnergyDipolesMACE on 8 TRN2 NeuronCores (v4).

Host (index-only prep): drop edges with r >= R_MAX (their messages are
exactly zero), then LPT-balance destination nodes into 64 windows of 128
nodes (and windows into cores) so the per-window edge cap is near the mean
(~8 chunks of 128 edges instead of the unbalanced 11). All per-node tensors
are permuted consistently; per-graph sums via one-hot make the permutation
transparent to the output.

Device per core: geometry once (Y, radial basis; bf16 outputs); rbw
transposed via idle HWDGE DMA-transposes; radial MLP for BOTH iterations
fused (128-partition block-diagonal weights) in 4-chunk groups; per chunk:
px matmul -> bf16 SBUF copy (Pool/Act) -> msg = yh3*pxb on DVE in 2x mode
-> transposed one-hot scatter (3 matmuls, A arrives (mc, node)-major so the
node phase needs no PE transpose + PSUM copies for the mix). Node-phase h
is bf16; its (mc, node) transpose for self-connection/readouts goes through
HWDGE DMA-transpose. Iteration-2 h flows through a per-window bf16
AllGather overlapped behind remaining it-0 compute + SWDGE re-gather.
"""
import math
import numpy as np

import concourse.bacc as bacc
import concourse.bass as bass
import concourse.tile as tile
from concourse import mybir

# allow 128B gather payloads (probed on HW previously)
import textwrap as _tw, inspect as _ins
try:
    _gsrc = _tw.dedent(_ins.getsource(bass.BassGpSimd.dma_gather))
except OSError:      # already patched by another module instance
    _gsrc = ""
if "% 256 == 0" in _gsrc:
    _gsrc = _gsrc.replace("elem_size_bytes > 0 and elem_size_bytes % 256 == 0",
                          "elem_size_bytes > 0 and elem_size_bytes % 128 == 0")
    _gns = dict(bass.__dict__)
    exec(compile(_gsrc, "<patched_dma_gather>", "exec"), _gns)
    bass.BassGpSimd.dma_gather = _gns["dma_gather"]

f32 = mybir.dt.float32
bf16 = mybir.dt.bfloat16
i16 = mybir.dt.int16
i32 = mybir.dt.int32
AF = mybir.ActivationFunctionType
ALU = mybir.AluOpType

N, E, C, Z, G, NB, NSH = 8192, 131072, 32, 10, 16, 8, 9
R_MAX, P_CUT, AVG_NEIGH = 5.0, 5, 16.0
LMAP = np.array([0, 1, 1, 1, 2, 2, 2, 2, 2])
NCORES = 8
NPC = N // NCORES
WIN = 128
WPC = NPC // WIN               # 8 windows/core
NWIN = NCORES * WPC            # 64 global windows
CHUNK = 128
NQ = 4                         # SWDGE queues
HR2 = 128                      # agout row elems (bf16; 256B step, 128B payload)
MC = NSH * C                   # 288
S3, S5, S15 = 3.0 ** 0.5, 5.0 ** 0.5, 15.0 ** 0.5
PREF = (2.0 / R_MAX) ** 0.5
PCF = float(P_CUT)
ENV_A = -(PCF + 1.0) * (PCF + 2.0) / 2.0
ENV_B = PCF * (PCF + 2.0)
ENV_C = -PCF * (PCF + 1.0) / 2.0
TWO_PI = 2 * math.pi

DEFAULT_W_CAP = 8              # balanced cap for reference.setup_inputs()
_DYN = {"W_CAP": DEFAULT_W_CAP}
GCH = 6                        # chunks per gather call / yh3 span


class Dims:
    def __init__(self, cap):
        self.W_CAP = cap
        self.L_PAD = WPC * cap * CHUNK
        self.NCHUNKS = WPC * cap
        self.IDX_COLS = self.L_PAD // 16
        self.RBCH = -(-self.NCHUNKS // 16) * 16   # rbw chunks padded to 16


def blob_layout(D):
    """(name, rows, shape-after-cols) per blob; device views slice rows/cols."""
    f32e = [("rcvloc", 128, (D.NCHUNKS,)), ("nvec", 128, (NB,)),
            ("ident", 128, (128,))]
    f32l = [("Wmix", 96, (2, 3, 96)), ("e0own", 128, (WPC,)),
            ("goh", 128, (WPC, G)), ("qown", 128, (WPC,)),
            ("posown", 128, (WPC, 3)), ("w123", 128, (WPC, 2, 3 * C))]
    bf = [("iota", 128, (128,)), ("R0blk", 128, (16, 128)),
          ("R1bd", 128, (128,)), ("R2bd", 128, (128,)), ("R3z", 128, (2, 3 * C)),
          ("Wsc", 96, (2, 3, 96)), ("Wh", C, (16,)), ("wE2", 16, (1,)),
          ("wE1r", 128, (C,)), ("wrd", 128, (2, 96)), ("h0oT", C, (WPC, 128))]
    return {"cstf32e": f32e, "cstf32l": f32l, "cstbf": bf}


def blob_cols(entries):
    return sum(int(np.prod(s)) for _, _, s in entries)


def const_specs(D):
    lay = blob_layout(D)
    return dict(
        cstf32e=([128, blob_cols(lay["cstf32e"])], f32),
        cstbf=([128, blob_cols(lay["cstbf"])], bf16),
        cstf32l=([128, blob_cols(lay["cstf32l"])], f32),
        gsnd=([128, D.IDX_COLS], i16), gsnd2=([128, D.IDX_COLS], i16),
    )


def big_input_specs(D):
    return dict(
        sndpos=([128, D.NCHUNKS, 3], f32), rcvpos=([128, D.NCHUNKS, 3], f32),
        hs0=([128, D.NCHUNKS, C], bf16),
    )


def _balance(deg):
    """LPT: nodes -> 64 windows of exactly 128 nodes; windows -> cores.
    Returns node_core, node_win, node_lane (int64 arrays [N])."""
    import heapq
    order = np.argsort(-deg, kind="stable")
    counts = np.zeros(NWIN, np.int64)
    loads = np.zeros(NWIN, np.int64)
    node_gw = np.empty(N, np.int64)
    node_lane = np.empty(N, np.int64)
    heap = [(0, w) for w in range(NWIN)]
    heapq.heapify(heap)
    for n in order:
        while True:
            load, w = heapq.heappop(heap)
            if counts[w] < WIN:
                break
        node_gw[n] = w
        node_lane[n] = counts[w]
        counts[w] += 1
        loads[w] += deg[n]
        if counts[w] < WIN:
            heapq.heappush(heap, (int(loads[w]), w))
    # windows -> cores (LPT, 8 windows per core)
    worder = np.argsort(-loads, kind="stable")
    ccnt = np.zeros(NCORES, np.int64)
    cload = np.zeros(NCORES, np.int64)
    win_core = np.empty(NWIN, np.int64)
    win_idx = np.empty(NWIN, np.int64)
    cheap = [(0, k) for k in range(NCORES)]
    heapq.heapify(cheap)
    for w in worder:
        while True:
            cl, k = heapq.heappop(cheap)
            if ccnt[k] < WPC:
                break
        win_core[w] = k
        win_idx[w] = ccnt[k]
        ccnt[k] += 1
        cload[k] += loads[w]
        if ccnt[k] < WPC:
            heapq.heappush(cheap, (int(cload[k]), k))
    node_core = win_core[node_gw]
    node_win = win_idx[node_gw]
    return node_core, node_win, node_lane, int(loads.max())


def host_prep(inputs):
    snd = np.asarray(inputs["edge_index"])[0].astype(np.int64)
    rcv = np.asarray(inputs["edge_index"])[1].astype(np.int64)
    batch = np.asarray(inputs["batch"]).astype(np.int64)
    positions = np.asarray(inputs["positions"], np.float32)
    node_attrs = np.asarray(inputs["node_attrs"], np.float32)
    charges = np.asarray(inputs["charges"], np.float32)

    # exact sparsity: r >= R_MAX edges have identically-zero messages
    dvec = positions[rcv].astype(np.float64) - positions[snd].astype(np.float64)
    keep = (dvec * dvec).sum(1) < R_MAX * R_MAX * (1 + 1e-6)
    snd, rcv = snd[keep], rcv[keep]

    deg = np.bincount(rcv, minlength=N)
    node_core, node_win, node_lane, maxload = _balance(deg)
    cap = max(1, -(-maxload // CHUNK))
    _DYN["W_CAP"] = cap
    D = Dims(cap)

    # agout row of node n under the two orderings
    row1 = node_win * NPC + node_core * WIN + node_lane   # agmode=1 (window-major)
    row0 = node_core * NPC + node_win * WIN + node_lane   # agmode=0 (core-major)
    own_node = np.empty((NCORES, WPC, WIN), np.int64)
    own_node[node_core, node_win, node_lane] = np.arange(N)

    # edges sorted into (core, window) buckets of their destination
    egw = node_core[rcv] * WPC + node_win[rcv]
    order = np.argsort(egw, kind="stable")
    snd_s, rcv_s, egw_s = snd[order], rcv[order], egw[order]
    counts = np.bincount(egw_s, minlength=NWIN)
    starts = np.zeros(NWIN + 1, np.int64)
    np.cumsum(counts, out=starts[1:])

    import ml_dtypes
    bfnp = ml_dtypes.bfloat16
    iota = np.tile(np.arange(128, dtype=np.float32)[None, :], (128, 1)).astype(bfnp)
    ident = np.eye(128, dtype=np.float32)
    nvec = np.tile((np.arange(1, NB + 1, dtype=np.float32) * math.pi / R_MAX)[None, :],
                   (128, 1))
    # MLP weights: both iterations stacked (64+64 features on 128 partitions)
    R0cat = np.concatenate([np.asarray(inputs["R0"][i], np.float32)
                            for i in range(2)], 1)            # [8, 128]
    # zero-padded 128-row stationary: variant v contracts rows v*8..v*8+8
    # (chunk v of a 16-chunk DMA-transposed group)
    R0blk = np.zeros((128, 16, 128), np.float32)
    for v in range(16):
        R0blk[v * 8:(v + 1) * 8, v, :] = R0cat
    R1bd = np.zeros((128, 128), np.float32)
    R2bd = np.zeros((128, 128), np.float32)
    for i in range(2):
        R1bd[i*64:(i+1)*64, i*64:(i+1)*64] = np.asarray(inputs["R1"][i], np.float32)
        R2bd[i*64:(i+1)*64, i*64:(i+1)*64] = np.asarray(inputs["R2"][i], np.float32)
    R3z = np.zeros((128, 2, 3 * C), np.float32)
    for i in range(2):
        R3z[i*64:(i+1)*64, i, :] = np.asarray(inputs["R3"][i], np.float32)

    def blockdiag(a):
        # a: [2, 9, C, C] -> [96, 2, 3, 96]: per iteration and m-group g,
        # block-diagonal of the group's three per-m CxC matrices
        out = np.zeros((96, 2, 3, 96), np.float32)
        for i in range(2):
            for mm in range(NSH):
                g, r = mm // 3, (mm % 3) * C
                out[r:r + C, i, g, r:r + C] = a[i, mm]
        return out

    Wmix = blockdiag(np.stack(
        [np.asarray(inputs["W_mix"][i], np.float32)[LMAP] for i in range(2)], 0))
    Wsc = blockdiag(np.stack(
        [np.asarray(inputs["W_sc"][i], np.float32)[LMAP] for i in range(2)], 0))
    Wh = np.asarray(inputs["Wh"], np.float32)                 # [C, 16]
    wE2 = np.asarray(inputs["wE2"], np.float32)[:, None]
    # readout weights replicated across partitions for DVE mult+reduce
    wE1r = np.tile(np.asarray(inputs["wE1"], np.float32)[None, :], (128, 1))
    wrd = np.stack([np.tile(np.asarray(inputs["wD1"], np.float32), 3),
                    np.tile(np.asarray(inputs["wD2"], np.float32), 3)], 0)
    wrd = np.tile(wrd[None], (128, 1, 1))                     # [128, 2, 96]

    # host index-prep: per-node species lookups (node_attrs is one-hot)
    h0full = node_attrs @ np.asarray(inputs["W_embed"], np.float32)   # [N, C]
    e0full = node_attrs @ np.asarray(inputs["atomic_energies"], np.float32)  # [N]
    wfull = [[node_attrs @ np.asarray(inputs[f"Wp{j}"], np.float32)[i]
              for j in (1, 2, 3)] for i in range(2)]          # [2][3] of [N, C]

    lay = blob_layout(D)
    shared_vals = dict(iota=iota, ident=ident, nvec=nvec,
                       R0blk=R0blk.astype(bfnp), R1bd=R1bd.astype(bfnp),
                       R2bd=R2bd.astype(bfnp), R3z=R3z.astype(bfnp), Wmix=Wmix,
                       Wsc=Wsc.astype(bfnp), Wh=Wh.astype(bfnp),
                       wE2=wE2.astype(bfnp), wE1r=wE1r.astype(bfnp),
                       wrd=wrd.astype(bfnp))

    def pack_blob(entries, vals, dt):
        cols = blob_cols(entries)
        blob = np.zeros((128, cols), dt)
        off = 0
        for name, rows, shape in entries:
            n = int(np.prod(shape))
            blob[0:rows, off:off + n] = np.asarray(vals[name], dt).reshape(rows, n)
            off += n
        return blob

    in_maps = []
    for k in range(NCORES):
        snd_pad = np.zeros(D.L_PAD, np.int64)
        rcv_pad = np.zeros(D.L_PAD, np.int64)
        rcv_lane = np.full(D.L_PAD, -1000.0, np.float32)
        for w in range(WPC):
            gw = k * WPC + w
            lo, hi = starts[gw], starts[gw + 1]
            cnt = int(hi - lo)
            base = w * cap * CHUNK
            snd_pad[base:base + cnt] = snd_s[lo:hi]
            rcv_pad[base:base + cnt] = rcv_s[lo:hi]
            rcv_lane[base:base + cnt] = node_lane[rcv_s[lo:hi]].astype(np.float32)

        def wrap_idx(a):
            w16 = a.astype(np.int16).reshape(D.IDX_COLS, 16).T
            return np.tile(w16, (8, 1)).copy()

        def edge_fmt(a, dt=np.float32):  # [L_PAD, d] -> [128, NCHUNKS, d]
            d = a.shape[1]
            return np.ascontiguousarray(
                a.reshape(D.NCHUNKS, CHUNK, d).transpose(1, 0, 2)).astype(dt)

        own = own_node[k].reshape(-1)            # [NPC] node ids (w-major)
        vals = dict(shared_vals)
        vals["rcvloc"] = np.ascontiguousarray(rcv_lane.reshape(D.NCHUNKS, CHUNK).T)
        vals["e0own"] = np.ascontiguousarray(e0full[own].reshape(WPC, 128).T)
        vals["w123"] = np.ascontiguousarray(
            np.stack([np.concatenate([wfull[i][j][own] for j in range(3)], 1)
                      for i in range(2)], 1)                  # [NPC, 2, 3C]
            .reshape(WPC, 128, 2, 3 * C).transpose(1, 0, 2, 3))
        vals["h0oT"] = np.ascontiguousarray(
            h0full[own].reshape(WPC, 128, C).transpose(2, 0, 1)).astype(bfnp)
        goh = np.zeros((NPC, G), np.float32)
        goh[np.arange(NPC), batch[own]] = 1.0
        vals["goh"] = np.ascontiguousarray(goh.reshape(WPC, 128, G).transpose(1, 0, 2))
        vals["qown"] = np.ascontiguousarray(charges[own].reshape(WPC, 128).T)
        vals["posown"] = np.ascontiguousarray(
            positions[own].reshape(WPC, 128, 3).transpose(1, 0, 2))
        m = {}
        m["cstf32e"] = pack_blob(lay["cstf32e"], vals, np.float32)
        m["cstf32l"] = pack_blob(lay["cstf32l"], vals, np.float32)
        m["cstbf"] = pack_blob(lay["cstbf"], vals, bfnp)
        m["gsnd"] = wrap_idx(row0[snd_pad])
        m["gsnd2"] = wrap_idx(row1[snd_pad])
        m["sndpos"] = edge_fmt(positions[snd_pad])
        m["rcvpos"] = edge_fmt(positions[rcv_pad])
        m["hs0"] = edge_fmt(h0full[snd_pad], bfnp)
        in_maps.append(m)
    return in_maps, {}


def build_nc(num_devices=NCORES, sim_safe=False, phases=99, repeat=1, agmode=0,
             w_cap=None, wkbufs=6, pxb_eng=(0, 0), yh3_eng=(1, 1), ohsb_eng=2,
             hwm_eng=(0, 0), stag=1, compact_ag=1, nq=4, **_kw):
    """pxb_eng/yh3_eng: per-iteration engine policy.
    pxb_eng[it]: 0=Act, 1=Pool, 2=alternate Act/Pool by chunk.
    yh3_eng[it]: 0=DVE, 1=Pool, 2=alternate DVE/Pool by span.
    ohsb_eng: 0=DVE, 1=Pool, 2=mostly-DVE (every 4th on Pool).
    hwm_eng[it]: 0=DVE, 1=Pool for the node-phase hw multiply."""
    D = Dims(w_cap if w_cap is not None else _DYN["W_CAP"])
    CSPEC, BSPEC = const_specs(D), big_input_specs(D)
    nc = bacc.Bacc("TRN2", target_bir_lowering=False, debug=False,
                   num_devices=num_devices, num_swdge_queues=nq)
    inp = {name: nc.dram_tensor(name, shape, dt, kind="ExternalInput")
           for name, (shape, dt) in {**CSPEC, **BSPEC}.items()}
    y_out = nc.dram_tensor("y", [G, 4], f32, kind="ExternalOutput")
    compact_ag = compact_ag and agmode == 0
    AGW = 64 if compact_ag else HR2    # collective row width (elems)
    agin = nc.dram_tensor("agin", [NPC, AGW], bf16, kind="Internal")
    agout = nc.dram_tensor("agout", [N, HR2], bf16, kind="Internal",
                           addr_space="Shared" if not compact_ag else "Local")
    agout2 = (nc.dram_tensor("agout2", [N, AGW], bf16, kind="Internal",
                             addr_space="Shared") if compact_ag else None)

    def silu(out_ap, in_ap, pool, tag="siltmp"):
        if not sim_safe:
            nc.scalar.activation(out_ap, in_ap, AF.Silu)
        else:
            sg = pool.tile(list(out_ap.shape), f32, tag=tag)
            nc.scalar.activation(sg[:], in_ap, AF.Sigmoid)
            nc.vector.tensor_tensor(out_ap, in_ap, sg[:], ALU.mult)

    def gather_h(dst_tile, src_dram, idx_tile):
        # call j covers chunks [j*6, j*6+6), queue j%NQ: consecutive calls
        # land on different queues so chunks are ready ~in order
        for j in range(-(-D.NCHUNKS // GCH)):
            b = j * GCH
            g = min(GCH, D.NCHUNKS - b)
            nc.gpsimd.dma_gather(
                out_ap=dst_tile[:, b:b + g, :],
                in_ap=src_dram.ap()[:, 0:64],
                idxs_ap=idx_tile[:, b * 8:(b + g) * 8],
                num_idxs=g * CHUNK, num_idxs_reg=g * CHUNK,
                elem_size=64, elem_step=HR2, queue_num=j % nq)

    with tile.TileContext(nc) as tc:
        with tc.tile_pool(name="const", bufs=1) as cst, \
             tc.tile_pool(name="big", bufs=1) as big, \
             tc.tile_pool(name="pmlp", bufs=2, space="PSUM") as pmlp, \
             tc.tile_pool(name="px", bufs=2, space="PSUM") as pxp, \
             tc.tile_pool(name="pa", bufs=2, space="PSUM") as pap, \
             tc.tile_pool(name="pmisc", bufs=1, space="PSUM") as pms:

            sb = {}
            lay = blob_layout(D)
            blob_eng = {"cstf32e": nc.sync, "cstbf": nc.scalar,
                        "cstf32l": nc.scalar}
            blobs = {}
            for name, (shape, dt) in CSPEC.items():
                if (name == "gsnd" and agmode == 1) or \
                   (name == "gsnd2" and agmode == 0):
                    continue
                t = cst.tile(shape, dt, tag=f"c_{name}")
                blob_eng.get(name, nc.scalar).dma_start(out=t[:], in_=inp[name].ap())
                blobs[name] = t
            for bname, entries in lay.items():
                off = 0
                for name, rows, shape in entries:
                    n = int(np.prod(shape))
                    v = blobs[bname][0:rows, off:off + n]
                    if len(shape) > 1:
                        pat = " ".join(f"d{i}" for i in range(len(shape)))
                        v = v.rearrange(f"p ({pat}) -> p {pat}",
                                        **{f"d{i}": int(shape[i])
                                           for i in range(len(shape))})
                    sb[name] = v
                    off += n
            if "gsnd" in blobs:
                sb["gsnd"] = blobs["gsnd"]
            if "gsnd2" in blobs:
                sb["gsnd2"] = blobs["gsnd2"]
            R0b, R1b, R2b = sb["R0blk"], sb["R1bd"], sb["R2bd"]
            R3zb, Wscb, Whb = sb["R3z"], sb["Wsc"], sb["Wh"]
            wE2b, wE1rb, wrdb = sb["wE2"], sb["wE1r"], sb["wrd"]
            h0oTb = sb["h0oT"]

            # persistent tiles
            hsE = big.tile([128, D.NCHUNKS, 64], bf16, tag="hsE")
            Ysb = big.tile([128, D.NCHUNKS, NSH], bf16, tag="Y")
            s3_all = big.tile([128, D.L_PAD], bf16, tag="s3_all")
            ohsb = big.tile([128, D.NCHUNKS, 128], bf16, tag="ohsb")
            rbwB = big.tile([128, D.RBCH, NB], bf16, tag="rbwB")
            rbT = big.tile([128, D.RBCH // 16, 128], bf16, tag="rbT")
            hT2 = big.tile([128, WPC, 3, 128], bf16, tag="hT2")
            e0_sb = sb["e0own"]
            vals = big.tile([128, WPC, 4], f32, tag="vals")

            for _rep in range(repeat):
              # ---- geometry: whole-tensor r/sqrt prologue (one Sqrt table
              # load, overlapped with const DMAs), then 16-chunk groups for
              # the Sin/envelope chain so the MLP can start early
              if phases >= 1:
                with tc.tile_pool(name="geos", bufs=2) as gsc, \
                     tc.tile_pool(name="geop", bufs=1) as gpp:
                    spos = gpp.tile([128, D.NCHUNKS, 3], f32, tag="spos")
                    nc.sync.dma_start(out=spos[:], in_=inp["sndpos"].ap())
                    rpos = gpp.tile([128, D.NCHUNKS, 3], f32, tag="rpos")
                    nc.sync.dma_start(out=rpos[:], in_=inp["rcvpos"].ap())
                    nc.sync.dma_start(out=hsE[:, :, 0:C], in_=inp["hs0"].ap())
                    nc.vector.memset(Ysb[:, :, 0].unsqueeze(2), 1.0)
                    if D.RBCH > D.NCHUNKS:
                        nc.vector.memset(rbwB[:, D.NCHUNKS:D.RBCH, :], 0.0)

                    rgeo = gpp.tile([128, D.NCHUNKS, 3], f32, tag="rgeo")
                    r_a = rgeo[:, :, 0].unsqueeze(2)
                    rinv_a = rgeo[:, :, 1].unsqueeze(2)
                    xx_a = rgeo[:, :, 2].unsqueeze(2)
                    uu = gpp.tile([128, D.NCHUNKS, 3], f32, tag="uu")
                    BC = [128, D.NCHUNKS, 3]
                    nc.vector.tensor_tensor(uu[:], rpos[:], spos[:], ALU.subtract)
                    sqv = gpp.tile([128, D.NCHUNKS, 3], f32, tag="sqv")
                    nc.vector.tensor_tensor(sqv[:], uu[:], uu[:], ALU.mult)
                    nc.vector.tensor_reduce(r_a, sqv[:], mybir.AxisListType.X, ALU.add)
                    nc.vector.tensor_scalar_add(r_a, r_a, 1e-12)
                    nc.scalar.activation(r_a, r_a, AF.Sqrt)
                    nc.vector.reciprocal(rinv_a, r_a)
                    nc.gpsimd.tensor_scalar(xx_a, r_a, 1.0 / R_MAX, None, ALU.mult)
                    nc.vector.tensor_tensor(uu[:], uu[:],
                                            rinv_a.broadcast_to(BC), ALU.mult)

                    for g in range(D.RBCH // 16):
                        c0 = 16 * g
                        cn = min(16, D.NCHUNKS - c0)
                        cs = slice(c0, c0 + cn)
                        r_ = rgeo[:, cs, 0].unsqueeze(2)
                        rinv = rgeo[:, cs, 1].unsqueeze(2)
                        xx = rgeo[:, cs, 2].unsqueeze(2)
                        geot = gsc.tile([128, 16, 6], f32, tag="geo")
                        geo = geot[:, 0:cn, :]
                        rbwt = gsc.tile([128, 16, NB], f32, tag="rbw")
                        rbw = rbwt[:, 0:cn, :]
                        # radial basis arg, range-reduced to [-pi, pi]
                        BC8 = [128, cn, NB]
                        nc.vector.tensor_tensor(rbw, r_.broadcast_to(BC8),
                                                sb["nvec"].unsqueeze(1).broadcast_to(BC8),
                                                ALU.mult)
                        rmskt = gsc.tile([128, 16, NB], f32, tag="rmsk")
                        rkit = gsc.tile([128, 16, NB], i32, tag="rki")
                        rmsk, rki = rmskt[:, 0:cn, :], rkit[:, 0:cn, :]
                        nc.scalar.activation(rmsk, rbw, AF.Copy, scale=1.0 / TWO_PI)
                        nc.vector.tensor_copy(rki, rmsk)
                        nc.vector.tensor_copy(rmsk, rki)
                        nc.vector.scalar_tensor_tensor(rbw, rmsk, -TWO_PI, rbw,
                                                       ALU.mult, ALU.add)
                        nc.vector.tensor_scalar(rmsk, rbw, math.pi, None, ALU.is_gt)
                        nc.vector.scalar_tensor_tensor(rbw, rmsk, -TWO_PI, rbw,
                                                       ALU.mult, ALU.add)
                        nc.gpsimd.tensor_scalar(rbw, rbw, math.pi, None, ALU.min)
                        nc.gpsimd.tensor_scalar(rbw, rbw, -math.pi, None, ALU.max)
                        nc.scalar.activation(rbw, rbw, AF.Sin)
                        # envelope
                        x2 = geo[:, :, 0].unsqueeze(2)
                        nc.scalar.square(x2, xx)
                        x4 = geo[:, :, 1].unsqueeze(2)
                        nc.scalar.square(x4, x2)
                        x5 = geo[:, :, 2].unsqueeze(2)
                        nc.gpsimd.tensor_tensor(x5, x4, xx, ALU.mult)
                        q1 = geo[:, :, 3].unsqueeze(2)
                        nc.scalar.activation(q1, xx, AF.Copy, scale=ENV_C, bias=ENV_B)
                        q2 = geo[:, :, 4].unsqueeze(2)
                        nc.gpsimd.tensor_tensor(q2, q1, xx, ALU.mult)
                        nc.gpsimd.tensor_scalar_add(q2, q2, ENV_A)
                        env = geo[:, :, 5].unsqueeze(2)
                        nc.gpsimd.tensor_tensor(env, x5, q2, ALU.mult)
                        nc.gpsimd.tensor_scalar_add(env, env, 1.0)
                        mlt = geo[:, :, 0].unsqueeze(2)
                        nc.vector.tensor_scalar(mlt, xx, 1.0, None, ALU.is_lt)
                        nc.gpsimd.tensor_tensor(env, env, mlt, ALU.mult)
                        wfac = geo[:, :, 1].unsqueeze(2)
                        nc.vector.scalar_tensor_tensor(wfac, rinv, PREF,
                                                       env, ALU.mult, ALU.mult)
                        nc.vector.tensor_tensor(rbwB[:, cs, :], rbw,
                                                wfac.broadcast_to(BC8), ALU.mult)
                        # transpose this group for the MLP (idle HWDGE)
                        nc.sync.dma_start_transpose(
                            rbT[:, g, :], rbwB[:, 16 * g:16 * g + 16, :])
                        # spherical harmonics for this group
                        u = uu[:, cs, :]
                        ux = u[:, :, 0].unsqueeze(2)
                        uy = u[:, :, 1].unsqueeze(2)
                        uz = u[:, :, 2].unsqueeze(2)
                        Ys = Ysb[:, cs, :]
                        nc.scalar.activation(Ys[:, :, 1:4], u, AF.Copy, scale=S3)
                        nc.vector.scalar_tensor_tensor(Ys[:, :, 4].unsqueeze(2), ux, S15,
                                                       uy, ALU.mult, ALU.mult)
                        nc.vector.scalar_tensor_tensor(Ys[:, :, 5].unsqueeze(2), uy, S15,
                                                       uz, ALU.mult, ALU.mult)
                        t0 = geo[:, :, 2].unsqueeze(2)
                        t1 = geo[:, :, 3].unsqueeze(2)
                        nc.gpsimd.tensor_tensor(t0, uz, uz, ALU.mult)
                        nc.scalar.activation(Ys[:, :, 6].unsqueeze(2), t0,
                                             AF.Copy, scale=3.0 * S5 / 2.0, bias=-S5 / 2.0)
                        nc.vector.scalar_tensor_tensor(Ys[:, :, 7].unsqueeze(2), ux, S15,
                                                       uz, ALU.mult, ALU.mult)
                        nc.gpsimd.tensor_tensor(t0, ux, uy, ALU.add)
                        nc.gpsimd.tensor_tensor(t1, ux, uy, ALU.subtract)
                        nc.vector.scalar_tensor_tensor(Ys[:, :, 8].unsqueeze(2),
                                                       t0, S15 / 2.0,
                                                       t1, ALU.mult, ALU.mult)
                        # one-hot blocks for this group
                        for ch in range(c0, c0 + cn):
                            eng = nc.gpsimd if (ohsb_eng == 1 or
                                                (ohsb_eng == 2 and ch % 4 == 3)) \
                                else nc.vector
                            eng.tensor_scalar(
                                ohsb[:, ch, :], sb["iota"][:],
                                sb["rcvloc"][:, ch].unsqueeze(1),
                                1.0 / AVG_NEIGH, ALU.is_equal, ALU.mult)

              # ---- iterations
              with tc.tile_pool(name="wk", bufs=wkbufs) as wk, \
                   tc.tile_pool(name="nd", bufs=2) as ndp:
                  nc.vector.memset(vals[:], 0.0)

                  def emit_mlp_group(g):
                      # chunks [4g, 4g+4): 8->64->64->64 for both iterations
                      p1 = pmlp.tile([128, 512], f32, tag="pmlp")
                      for j in range(4):
                          ch = 4 * g + j
                          nc.tensor.matmul(p1[:, j * 128:(j + 1) * 128],
                                           R0b[:, ch % 16, :], rbT[:, ch // 16, :],
                                           start=True, stop=True)
                      s1 = wk.tile([128, 512], bf16, tag="s1")
                      silu(s1[:], p1[:], wk)
                      p2 = pmlp.tile([128, 512], f32, tag="pmlp")
                      nc.tensor.matmul(p2[:], R1b[:], s1[:], start=True, stop=True)
                      s2 = wk.tile([128, 512], bf16, tag="s2")
                      silu(s2[:], p2[:], wk)
                      p3 = pmlp.tile([128, 512], f32, tag="pmlp")
                      nc.tensor.matmul(p3[:], R2b[:], s2[:], start=True, stop=True)
                      ee = 4 * g * CHUNK
                      silu(s3_all[:, ee:ee + 512], p3[:], wk)

                  def emit_chunk(it, ch, w):
                      first = (ch == w * D.W_CAP)
                      last = (ch == (w + 1) * D.W_CAP - 1)
                      if first:
                          pA = pap.tile([96, 3, 128], f32, tag="pA")
                          emit_chunk.pA = pA
                      pA = emit_chunk.pA
                      px = pxp.tile([128, 3 * C], f32, tag="px")
                      nc.tensor.matmul(px[:], s3_all[:, ch * 128:(ch + 1) * 128],
                                       R3zb[:, it, :], start=True, stop=True)
                      pxb = wk.tile([128, 3, C], bf16, tag="pxb")
                      nc.scalar.activation(pxb[:], px[:].rearrange(
                          "p (l c) -> p l c", l=3), AF.Copy)
                      # yh3 span (m=1..8 only; m=0 has Y=1 so msg reuses hsE)
                      if ch % GCH == 0:
                          spn = min(GCH, D.NCHUNKS - ch)
                          yh3 = wk.tile([128, GCH, NSH - 1, C], bf16, tag="yh3")
                          ye = yh3_eng[it]
                          yeng = nc.gpsimd if (ye == 1 or
                                               (ye == 2 and (ch // GCH) % 2)) else nc.vector
                          yeng.tensor_tensor(
                              yh3[:, 0:spn, :, :],
                              Ysb[:, ch:ch + spn, 1:NSH].unsqueeze(3)
                                  .broadcast_to([128, spn, NSH - 1, C]),
                              hsE[:, ch:ch + spn, 0:C].unsqueeze(2)
                                  .broadcast_to([128, spn, NSH - 1, C]),
                              ALU.mult)
                          emit_chunk.yh3 = yh3
                          emit_chunk.y0 = ch
                      msg = wk.tile([128, MC], bf16, tag="msg")
                      y3 = emit_chunk.yh3[:, ch - emit_chunk.y0, :, :]
                      mv = msg[:].rearrange("p (m c) -> p m c", m=NSH)
                      nc.vector.tensor_tensor(mv[:, 0:1, :],
                                              hsE[:, ch, 0:C].unsqueeze(1),
                                              pxb[:, 0:1, :], ALU.mult)
                      nc.vector.tensor_tensor(
                          mv[:, 1:4, :], y3[:, 0:3, :],
                          pxb[:, 1, :].unsqueeze(1).broadcast_to([128, 3, C]),
                          ALU.mult)
                      nc.vector.tensor_tensor(
                          mv[:, 4:9, :], y3[:, 3:8, :],
                          pxb[:, 2, :].unsqueeze(1).broadcast_to([128, 5, C]),
                          ALU.mult)
                      # transposed scatter: pA[mcgrp, j, n] += msg[e, mc]^T oh[e, n]
                      for j in range(3):
                          nc.tensor.matmul(pA[:, j, :], msg[:, j * 96:(j + 1) * 96],
                                           ohsb[:, ch, :], start=first, stop=last)
                      return pA

                  def emit_node_phase(it, w, pA):
                      # self-connection first: its inputs are ready long before
                      # the mix chain, so PE starts it while ATsb copies
                      psc = pms.tile([128, MC], f32, tag="psc")
                      if it == 0:
                          nc.tensor.matmul(psc[:, 0:C], h0oTb[:, w, :],
                                           Wscb[0:32, 0, 0, 0:C], start=True, stop=True)
                      else:
                          for g in range(3):
                              nc.tensor.matmul(psc[:, g * 96:(g + 1) * 96],
                                               hT2[0:96, w, g, :],
                                               Wscb[:, 1, g, :], start=True, stop=True)
                      ATsb = ndp.tile([96, 3, 128], f32, tag="ATsb")
                      nc.scalar.activation(ATsb[:], pA[:], AF.Copy)
                      pA2 = pms.tile([128, MC], f32, tag="pA2")
                      for g in range(3):
                          nc.tensor.matmul(pA2[:, g * 96:(g + 1) * 96],
                                           ATsb[:, g, :], sb["Wmix"][:, it, g, :],
                                           start=True, stop=True)
                      wslc = sb["w123"][:, w, it, :]
                      F = ndp.tile([128, C], f32, tag="F")
                      nc.vector.tensor_tensor(F[:], wslc[:, 2 * C:3 * C],
                                              pA2[:, 0:C], ALU.mult)
                      nc.vector.tensor_tensor(F[:], F[:], wslc[:, C:2 * C], ALU.add)
                      nc.vector.tensor_tensor(F[:], F[:], pA2[:, 0:C], ALU.mult)
                      nc.vector.tensor_tensor(F[:], F[:], wslc[:, 0:C], ALU.add)
                      # 320 cols: 288 real + pad so each 96-col group can be
                      # transposed via an overlapping 128-col XBAR window
                      hw_t = ndp.tile([128, 320], bf16, tag="hw")
                      heng = nc.gpsimd if hwm_eng[it] else nc.vector
                      heng.tensor_tensor(
                          hw_t[:, 0:MC].rearrange("p (m c) -> p m c", m=NSH),
                          pA2[:].rearrange("p (m c) -> p m c", m=NSH),
                          F[:].unsqueeze(1).broadcast_to([128, NSH, C]), ALU.mult)
                      if it == 0:
                          nc.vector.tensor_tensor(hw_t[:, 0:C], hw_t[:, 0:C],
                                                  psc[:, 0:C], ALU.add)
                          nc.vector.memset(hw_t[:, MC:320], 0.0)
                      else:
                          heng.tensor_tensor(hw_t[:, 0:MC], hw_t[:, 0:MC],
                                             psc[:], ALU.add)
                      # transposes of h (HWDGE): group j rows 0:96 = m 3j..3j+2
                      # (rows 96:128 alias the next group's first cols; unread).
                      # it=0 feeds it=1 self-connection (slack: half an
                      # iteration); it=1 only group 0 for the energy head.
                      ngrp = 3 if it == 0 else 1
                      for j in range(ngrp):
                          nc.sync.dma_start_transpose(
                              hT2[:, w, j, :], hw_t[:, j * 96:j * 96 + 128])
                      # dipole readout on DVE: d[n,m] = sum_c h[n,m,c]*wD[c]
                      dtmp = ndp.tile([128, 3, C], bf16, tag="dtmp")
                      nc.vector.tensor_tensor(dtmp[:], hw_t[:, C:4 * C].rearrange(
                          "p (m c) -> p m c", m=3), wrdb[:, it, :].rearrange(
                          "p (m c) -> p m c", m=3), ALU.mult)
                      dred = ndp.tile([128, 3], f32, tag="dred")
                      nc.vector.tensor_reduce(dred[:].unsqueeze(2), dtmp[:],
                                              mybir.AxisListType.X, ALU.add)
                      if it == 0:
                          nc.sync.dma_start(
                              out=agin.ap()[w * 128:(w + 1) * 128, 0:C],
                              in_=hw_t[:, 0:C])
                          # energy readout on DVE: e = sum_c h0[n,c]*wE1[c]
                          etmp = ndp.tile([128, C], bf16, tag="etmp")
                          nc.vector.tensor_tensor(etmp[:], hw_t[:, 0:C],
                                                  wE1rb[:], ALU.mult)
                          ered = ndp.tile([128, 1], f32, tag="ered")
                          nc.vector.tensor_reduce(ered[:].unsqueeze(2),
                                                  etmp[:].unsqueeze(1),
                                                  mybir.AxisListType.X, ALU.add)
                          nc.vector.tensor_tensor(vals[:, w, 0].unsqueeze(1),
                                                  ered[:], e0_sb[:, w].unsqueeze(1),
                                                  ALU.add)
                          nc.scalar.activation(vals[:, w, 1:4], dred[:], AF.Copy)
                      else:
                          phid = pmlp.tile([128, 16], f32, tag="pmlp")
                          nc.tensor.matmul(phid[:], hT2[0:32, w, 0, :], Whb[:],
                                           start=True, stop=True)
                          hid = ndp.tile([128, 16], f32, tag="hid")
                          silu(hid[:], phid[:], ndp)
                          pht = pmlp.tile([16, 128], f32, tag="pmlp")
                          nc.tensor.transpose(pht[:], hid[:], sb["ident"][:])
                          hidT = ndp.tile([16, 128], bf16, tag="hidT")
                          nc.scalar.activation(hidT[:], pht[:], AF.Copy)
                          prd = pmlp.tile([128, 1], f32, tag="pmlp")
                          nc.tensor.matmul(prd[:], hidT[:], wE2b[:],
                                           start=True, stop=True)
                          nc.vector.tensor_tensor(vals[:, w, 0].unsqueeze(1),
                                                  vals[:, w, 0].unsqueeze(1),
                                                  prd[:], ALU.add)
                          nc.vector.tensor_tensor(vals[:, w, 1:4], vals[:, w, 1:4],
                                                  dred[:], ALU.add)
                          nc.vector.scalar_tensor_tensor(
                              vals[:, w, 1:4], sb["posown"][:, w, :],
                              sb["qown"][:, w].unsqueeze(1), vals[:, w, 1:4],
                              ALU.mult, ALU.add)

                  def post_node(it, w):
                      # per-window AllGather (agmode=1): hide all but the
                      # last collective behind remaining it=0 compute
                      if it == 0 and phases >= 3 and num_devices > 1 and agmode == 1:
                          aout = agout.ap()[w * NPC:(w + 1) * NPC, :]
                          nc.gpsimd.collective_compute(
                              "AllGather", ALU.bypass,
                              replica_groups=[list(range(num_devices))],
                              ins=[agin.ap()[w * WIN:(w + 1) * WIN, :]],
                              outs=[aout])

                  STAG = stag
                  for it in range(2 if phases >= 3 else (1 if phases >= 2 else 0)):
                      pend = {}
                      for ch in range(D.NCHUNKS):
                          if it == 0 and ch % 4 == 0:
                              emit_mlp_group(ch // 4)
                          w = ch // D.W_CAP
                          pA = emit_chunk(it, ch, w)
                          if ch == (w + 1) * D.W_CAP - 1:
                              pend[w] = pA
                              if w >= STAG:
                                  emit_node_phase(it, w - STAG, pend.pop(w - STAG))
                                  post_node(it, w - STAG)
                      for w in range(WPC - STAG, WPC):
                          emit_node_phase(it, w, pend.pop(w))
                          post_node(it, w)

                      if it == 0 and phases >= 3:
                          if num_devices > 1:
                              if agmode == 0:
                                  aout = (agout2.ap() if compact_ag
                                          else agout.ap())
                                  nc.gpsimd.collective_compute(
                                      "AllGather", ALU.bypass,
                                      replica_groups=[list(range(num_devices))],
                                      ins=[agin.ap()], outs=[aout])
                                  if compact_ag:
                                      # local re-stride to 256B rows for SWDGE
                                      # (two DMAs -> two engines in parallel)
                                      H = N // 2
                                      nc.sync.dma_start(
                                          out=agout.ap()[0:H, 0:AGW],
                                          in_=agout2.ap()[0:H, :])
                                      nc.scalar.dma_start(
                                          out=agout.ap()[H:N, 0:AGW],
                                          in_=agout2.ap()[H:N, :])
                                  gather_h(hsE, agout, sb["gsnd"])
                              else:
                                  gather_h(hsE, agout, sb["gsnd2"])
                          else:
                              nc.sync.dma_start(out=hsE[:, :, 0:C], in_=inp["hs0"].ap())

                  # final reduction
                  pO = pmlp.tile([G, 4], f32, tag="pmlp")
                  if phases < 3:
                      for w in range(WPC):
                          nc.vector.scalar_tensor_tensor(
                              vals[:, w, 1:4], sb["posown"][:, w, :],
                              sb["qown"][:, w].unsqueeze(1), vals[:, w, 1:4],
                              ALU.mult, ALU.add)
                  for w in range(WPC):
                      nc.tensor.matmul(pO[:], sb["goh"][:, w, :], vals[:, w, :],
                                       start=(w == 0), stop=(w == WPC - 1))
                  y_sb = ndp.tile([G, 4], f32, tag="ysb")
                  nc.scalar.activation(y_sb[:], pO[:], AF.Copy)
                  nc.sync.dma_start(out=y_out.ap(), in_=y_sb[:])

    nc.compile()
    return nc


from concourse.bass_utils import run_bass_kernel_spmd as _run_spmd

_NC_CACHE = {}


def _get_nc():
    key = ("nc", _DYN["W_CAP"])
    if key not in _NC_CACHE:
        _NC_CACHE[key] = build_nc(num_devices=NCORES, sim_safe=False)
    return _NC_CACHE[key]


def kernel(**inputs):
    np_inputs = {k: np.asarray(v) for k, v in inputs.items()}
    in_maps, _ = host_prep(np_inputs)
    nc = _get_nc()
    res = _run_spmd(nc, in_maps, core_ids=list(range(NCORES)))
    y = sum(np.asarray(res.results[k]["y"], dtype=np.float64)
            for k in range(NCORES))
    return y.astype(np.float32)
